# revision 1
# baseline (speedup 1.0000x reference)
"""Trainium2 Bass kernel for nn_NestedParallelBlock.

Strategy: data-parallel over batch (core b <- batch b). Host sorts tokens by
expert into uniform-capacity groups (capacities = max count over the 8
batches, so the SPMD program is identical on every core); the nested feature
masks then become static K-tile / M-tile range restrictions (no mask tensors,
~47% FLOP reduction on both big matmuls). Attention is permutation-equivariant
so it runs in sorted order; dummy padding tokens are zero vectors whose k/v
rows vanish, handled exactly via a constant row-sum correction. All matmuls in
bf16 with f32 PSUM accumulation.
"""

import sys

if "/opt/trn_rl_repo" not in sys.path:
    sys.path.insert(0, "/opt/trn_rl_repo")

import numpy as np
import ml_dtypes

import concourse.bass as bass
import concourse.tile as tile
from concourse import bacc, mybir
from concourse.alu_op_type import AluOpType
from concourse.bass_utils import run_bass_kernel_spmd

BF16 = ml_dtypes.bfloat16
F32 = mybir.dt.float32
BF = mybir.dt.bfloat16
AF = mybir.ActivationFunctionType

DIM = 1024
NUM_EXPERTS = 4
NUM_HEADS = 16
DH = 64
MLP = 4096
EXPAND = 3 * DIM + MLP  # 7168
CDIM = 2 * DIM  # 2048
CKDIM = MLP + DIM  # 5120
B, N = 8, 1024
EPS = 1e-5

FLAGS = dict(
    packed_pairs=True,   # row-tiled packed S^T matmuls (2 heads concurrently)
    gpsimd_recip=False,  # offload rowsum bias+recip to GpSimd
)


def _build(C, flags):
    """Build the SPMD Tile program. C = per-expert group capacities (len 4)."""
    off = [0]
    for c in C:
        off.append(off[-1] + c)
    T = off[-1]
    TT = (T + 127) // 128
    Tpad = TT * 128
    n_dummy = float(T - N)
    # expert (group id) of each sorted position; uniform across cores
    grp_of = np.zeros(T, np.int64)
    for g in range(4):
        grp_of[off[g]:off[g + 1]] = g
    # k-tiles of xnT needed per token-tile = 2^(max group overlapping tile)
    ktiles_tt = [1 << int(grp_of[min(128 * (tt + 1), T) - 1]) for tt in range(TT)]
    # query blocks (512-aligned for PSUM banks)
    qblocks = []
    q0 = 0
    while q0 < T:
        nq = min(512, T - q0)
        qblocks.append((q0, nq))
        q0 += nq
    # contract valid-column start per cy M-tile
    # m 0..7: mlp rows (d_out >= 128(m+1)); m 8..15: attn rows (expert 3 only)
    def mstart(m):
        if m >= 8:
            return off[3]
        need = 128 * (m + 1)
        for g in range(4):
            if 256 << g >= need:
                return off[g]
        raise AssertionError

    nc = bacc.Bacc("TRN2", target_bir_lowering=False, debug=False, num_devices=8)

    x_s = nc.dram_tensor("x_s", [Tpad, DIM], F32, kind="ExternalInput").ap()
    xT_s = nc.dram_tensor("xT_s", [DIM, Tpad], F32, kind="ExternalInput").ap()
    probs_s = nc.dram_tensor("probs_s", [Tpad], F32, kind="ExternalInput").ap()
    we_t = nc.dram_tensor("we_t", [8, 56, 128, 128], BF, kind="ExternalInput").ap()
    wc_t = nc.dram_tensor("wc_t", [40, 16, 128, 128], BF, kind="ExternalInput").ap()
    qT_dram = nc.dram_tensor("qT_dram", [8, 128, Tpad], BF).ap()
    outT = nc.dram_tensor("outT", [DIM, Tpad], F32, kind="ExternalOutput").ap()

    x_r = x_s.rearrange("(tt p) d -> tt p d", p=128)
    xT_r = xT_s.rearrange("(f p) t -> f p t", p=128)
    outT_r = outT.rearrange("(f p) t -> f p t", p=128)

    with tile.TileContext(nc) as tc:
        # ---------------- persistent pool (LIFO outermost) ----------------
        pA = tc.alloc_tile_pool(name="persist", bufs=1)
        gT = pA.tile([128, 32, T], BF)          # gelu(mlp_hidden)^T
        kT = pA.tile([128, 8, Tpad], BF)        # k^T (post-LN)
        Vext = pA.tile([128, TT, 16, 65], BF)   # [V | 1] per head, token-major
        attnT = pA.tile([128, 8, T], BF)        # attention output^T
        probs_b = pA.tile([128, Tpad], F32)     # probs broadcast to 128 parts

        nc.sync.dma_start(
            out=probs_b,
            in_=bass.AP(tensor=probs_s.tensor, offset=probs_s.offset,
                        ap=[[0, 128]] + probs_s.ap),
        )
        nc.vector.memset(Vext[:, :, :, 64:65], 1.0)
        eps_t = pA.tile([128, 1], F32)
        nc.vector.memset(eps_t, EPS)

        # ---------------- phase 1+2+3 pool ----------------
        pB = tc.alloc_tile_pool(name="ph123", bufs=1)
        xnT = pB.tile([128, 8, Tpad], BF)       # masked-LN(x)^T
        kv_pre = pB.tile([128, TT, 2048], BF)   # pre-LN kv, token-major
        pBx = tc.alloc_tile_pool(name="ph1x", bufs=2)
        pBw = tc.alloc_tile_pool(name="ph2w", bufs=8)
        pBs = tc.alloc_tile_pool(name="ph2s", bufs=2)
        pBst = tc.alloc_tile_pool(name="ph1st", bufs=4)
        psE = tc.alloc_tile_pool(name="psE", bufs=2, space="PSUM")

        # ---- Phase 1: LN1 + transpose (token-tile loop) ----
        for tt in range(TT):
            x_t = pBx.tile([128, DIM], F32, tag="x")
            nc.sync.dma_start(out=x_t, in_=x_r[tt])
            st = pBst.tile([128, 2, 6], F32, tag="st")
            nc.vector.bn_stats(out=st[:, 0], in_=x_t[:, 0:512])
            nc.vector.bn_stats(out=st[:, 1], in_=x_t[:, 512:1024])
            mv = pBst.tile([128, 2], F32, tag="mv")
            nc.vector.bn_aggr(out=mv, in_=st)
            rstd = pBst.tile([128, 1], F32, tag="rstd")
            nc.scalar.activation(out=rstd, in_=mv[:, 1:2], func=AF.Ln, bias=eps_t)
            nc.scalar.activation(out=rstd, in_=rstd, func=AF.Exp, scale=-0.5)
            xn_t = pBx.tile([128, DIM], BF, tag="xn")
            nc.vector.tensor_scalar(
                out=xn_t, in0=x_t, scalar1=mv[:, 0:1], scalar2=rstd,
                op0=AluOpType.subtract, op1=AluOpType.mult)
            for k in range(ktiles_tt[tt]):
                nc.sync.dma_start_transpose(
                    out=xnT[:, k, 128 * tt:128 * (tt + 1)],
                    in_=xn_t[:, 128 * k:128 * (k + 1)])

        # ---- Phase 2: expand matmul ----
        # order: kv rows (8..23) then q rows (0..7) then mlp rows (24..55)
        # so attention can start while mlp-expand still runs.
        def expand_m(m):
            wts = []
            for k in range(8):
                w = pBw.tile([128, 128], BF, tag="we")
                nc.sync.dma_start(out=w, in_=we_t[k, m])
                wts.append(w)
            outs = {}
            for g in range(4):
                kt_g = 1 << g
                ps = psE.tile([128, C[g]], F32, tag=f"eps{g}")
                for k in range(kt_g):
                    nc.tensor.matmul(
                        ps, wts[k], xnT[:, k, off[g]:off[g + 1]],
                        start=(k == 0), stop=(k == kt_g - 1))
                outs[g] = ps
            return outs

        for m in range(8, 24):  # kv rows
            outs = expand_m(m)
            stage = pBs.tile([128, Tpad], BF, tag="kvstage")
            nc.vector.memset(stage[:, T:Tpad], 0.0)
            for g in range(4):
                nc.vector.tensor_copy(out=stage[:, off[g]:off[g + 1]], in_=outs[g])
            for tt in range(TT):
                nc.sync.dma_start_transpose(
                    out=kv_pre[:, tt, 128 * (m - 8):128 * (m - 7)],
                    in_=stage[:, 128 * tt:128 * (tt + 1)])

        for m in range(0, 8):  # q rows
            outs = expand_m(m)
            stage = pBs.tile([128, Tpad], BF, tag="kvstage")
            for g in range(4):
                nc.vector.tensor_copy(out=stage[:, off[g]:off[g + 1]], in_=outs[g])
            nc.sync.dma_start(out=qT_dram[m][:, 0:T], in_=stage[:, 0:T])

        # ---- Phase 3: kv layernorm (interleaved in program order; deps gate) ----
        for tt in range(TT):
            row = kv_pre[:, tt, :]
            st = pBst.tile([128, 4, 6], F32, tag="st2")
            for c in range(4):
                nc.vector.bn_stats(out=st[:, c], in_=row[:, 512 * c:512 * (c + 1)])
            mv = pBst.tile([128, 2], F32, tag="mv2")
            nc.vector.bn_aggr(out=mv, in_=st)
            rstd = pBst.tile([128, 1], F32, tag="rstd2")
            nc.scalar.activation(out=rstd, in_=mv[:, 1:2], func=AF.Ln, bias=eps_t)
            nc.scalar.activation(out=rstd, in_=rstd, func=AF.Exp, scale=-0.5)
            kn_t = pBx.tile([128, DIM], BF, tag="kn")
            nc.vector.tensor_scalar(
                out=kn_t, in0=row[:, 0:1024], scalar1=mv[:, 0:1], scalar2=rstd,
                op0=AluOpType.subtract, op1=AluOpType.mult)
            nc.vector.tensor_scalar(
                out=Vext[:, tt, :, 0:64],
                in0=row[:, 1024:2048], scalar1=mv[:, 0:1], scalar2=rstd,
                op0=AluOpType.subtract, op1=AluOpType.mult)
            for j in range(8):
                nc.sync.dma_start_transpose(
                    out=kT[:, j, 128 * tt:128 * (tt + 1)],
                    in_=kn_t[:, 128 * j:128 * (j + 1)])

        # mlp rows of expand (gelu fused in epilogue)
        for m in range(24, 56):
            outs = expand_m(m)
            for g in range(4):
                nc.scalar.activation(
                    out=gT[:, m - 24, off[g]:off[g + 1]], in_=outs[g], func=AF.Gelu)

        pBst.release()
        pBs.release()
        pBw.release()
        pBx.release()
        pB.release()
        psE.release()

        # ---------------- Phase 4: attention ----------------
        pC = tc.alloc_tile_pool(name="attn", bufs=2)
        pCq = tc.alloc_tile_pool(name="attnq", bufs=6)
        pCst = tc.alloc_tile_pool(name="attnst", bufs=2)
        psS = tc.alloc_tile_pool(name="psS", bufs=2, space="PSUM")
        psAV = tc.alloc_tile_pool(name="psAV", bufs=2, space="PSUM")

        rec_eng = nc.gpsimd if flags["gpsimd_recip"] else nc.vector

        for j in range(8):
            qts = []
            for (q0, nq) in qblocks:
                qt = pCq.tile([128, 512], BF, tag="qts")
                nc.sync.dma_start(out=qt[:, 0:nq], in_=qT_dram[j][:, q0:q0 + nq])
                qts.append(qt)
            expS0 = pC.tile([128, 9, T], BF, tag="expS")
            expS1 = pC.tile([128, 9, T], BF, tag="expS")
            expS = [expS0, expS1]
            for kt in range(TT):
                Mkt = min(128, T - 128 * kt)
                for h2 in range(2):
                    ps = psS.tile([128, T], F32, tag="S")
                    for qi, (q0, nq) in enumerate(qblocks):
                        lhsT = kT[64 * h2:64 * (h2 + 1), j, 128 * kt:128 * kt + Mkt]
                        rhs = qts[qi][64 * h2:64 * (h2 + 1), 0:nq]
                        if flags["packed_pairs"]:
                            nc.tensor.matmul(ps[0:Mkt, q0:q0 + nq], lhsT, rhs,
                                             tile_position=(64 * h2, 0))
                        else:
                            nc.tensor.matmul(ps[0:Mkt, q0:q0 + nq], lhsT, rhs)
                    nc.scalar.activation(
                        out=expS[h2][0:Mkt, kt, 0:T], in_=ps[0:Mkt, 0:T],
                        func=AF.Exp, scale=float(DH) ** -0.5)
            for h2 in range(2):
                h = 2 * j + h2
                srow = pCst.tile([1, Tpad], F32, tag="srow")
                recb = pCst.tile([64, Tpad], F32, tag="recb")
                for qi, (q0, nq) in enumerate(qblocks):
                    ps = psAV.tile([65, 512], F32, tag="AV")
                    for kt in range(TT):
                        Kkt = min(128, T - 128 * kt)
                        nc.tensor.matmul(
                            ps[:, 0:nq],
                            Vext[0:Kkt, kt, h, 0:65],
                            expS[h2][0:Kkt, kt, q0:q0 + nq],
                            start=(kt == 0), stop=(kt == TT - 1))
                    # rowsum -> SBUF, subtracting the uniform dummy-key count
                    nc.vector.tensor_scalar_add(
                        out=srow[:, q0:q0 + nq], in0=ps[64:65, 0:nq],
                        scalar1=-n_dummy)
                    rec_eng.reciprocal(out=srow[:, q0:q0 + nq],
                                       in_=srow[:, q0:q0 + nq])
                    nc.gpsimd.partition_broadcast(
                        recb[:, q0:q0 + nq], srow[:, q0:q0 + nq])
                    if h2 == 0:
                        nc.vector.tensor_mul(
                            out=attnT[0:64, j, q0:q0 + nq],
                            in0=ps[0:64, 0:nq], in1=recb[:, q0:q0 + nq])
                    else:
                        if qi == 0:
                            astage = pCst.tile([64, T], BF, tag="astage")
                        nc.vector.tensor_mul(
                            out=astage[:, q0:q0 + nq],
                            in0=ps[0:64, 0:nq], in1=recb[:, q0:q0 + nq])
                if h2 == 1:
                    nc.sync.dma_start(out=attnT[64:128, j, :], in_=astage)

        pCst.release()
        pCq.release()
        pC.release()
        psAV.release()
        psS.release()

        # ---------------- Phase 5: contract + combine ----------------
        pD = tc.alloc_tile_pool(name="contr", bufs=2)
        pDw = tc.alloc_tile_pool(name="contrw", bufs=8)
        psM = tc.alloc_tile_pool(name="psM", bufs=2, space="PSUM")
        psA2 = tc.alloc_tile_pool(name="psA2", bufs=2, space="PSUM")

        tA = off[3]
        for f in range(8):
            tM = mstart(f)
            L, La = T - tM, T - tA
            psm = psM.tile([128, T - mstart(0)], F32, tag="CM")
            psa = psA2.tile([128, La], F32, tag="CA")
            for k in range(40):
                rhs_full = gT[:, k, :] if k < 32 else attnT[:, k - 32, :]
                wm = pDw.tile([128, 128], BF, tag="wc")
                nc.sync.dma_start(out=wm, in_=wc_t[k, f])
                wa = pDw.tile([128, 128], BF, tag="wc")
                nc.sync.dma_start(out=wa, in_=wc_t[k, f + 8])
                c0 = 0
                while c0 < L:
                    ncol = min(512, L - c0)
                    nc.tensor.matmul(psm[:, c0:c0 + ncol], wm,
                                     rhs_full[:, tM + c0:tM + c0 + ncol],
                                     start=(k == 0), stop=(k == 39))
                    c0 += ncol
                c0 = 0
                while c0 < La:
                    ncol = min(512, La - c0)
                    nc.tensor.matmul(psa[:, c0:c0 + ncol], wa,
                                     rhs_full[:, tA + c0:tA + c0 + ncol],
                                     start=(k == 0), stop=(k == 39))
                    c0 += ncol
            xt = pD.tile([128, Tpad], F32, tag="xt")
            nc.sync.dma_start(out=xt, in_=xT_r[f])
            out_t = pD.tile([128, Tpad], F32, tag="out")
            nc.vector.tensor_mul(out=out_t[:, 0:T], in0=xt[:, 0:T],
                                 in1=probs_b[:, 0:T])
            nc.vector.tensor_add(out=out_t[:, 0:T], in0=out_t[:, 0:T],
                                 in1=xt[:, 0:T])
            tmp = pD.tile([128, T - mstart(0)], F32, tag="tmp")
            nc.vector.tensor_mul(out=tmp[:, 0:L], in0=psm[:, 0:L],
                                 in1=probs_b[:, tM:T])
            nc.vector.tensor_add(out=out_t[:, tM:T], in0=out_t[:, tM:T],
                                 in1=tmp[:, 0:L])
            nc.vector.tensor_add(out=out_t[:, tA:T], in0=out_t[:, tA:T],
                                 in1=psa[:, 0:La])
            nc.sync.dma_start(out=outT_r[f][:, 0:T], in_=out_t[:, 0:T])

        psA2.release()
        psM.release()
        pDw.release()
        pD.release()
        pA.release()

    nc.compile()
    return nc, T, Tpad


_CACHE = {}


def _get_program(C, flags_key):
    key = (tuple(C), flags_key)
    if key not in _CACHE:
        _CACHE[key] = _build(list(C), FLAGS)
    return _CACHE[key]


def kernel(**inputs):
    x = np.ascontiguousarray(np.asarray(inputs["x"], np.float32))
    em = np.asarray(inputs["expert_mask"]).astype(np.int64)
    ep = np.ascontiguousarray(np.asarray(inputs["expert_probs"], np.float32))
    We = np.asarray(inputs["expand_weight"], np.float32)
    Wc = np.asarray(inputs["contract_weight"], np.float32)
    mlp_bias = np.asarray(inputs["mlp_bias"], np.float32)
    cb = np.asarray(inputs["contract_bias"], np.float32)
    n1w = np.asarray(inputs["norm1_w"], np.float32)
    n1b = np.asarray(inputs["norm1_b"], np.float32)
    n2w = np.asarray(inputs["norm2_w"], np.float32)
    n2b = np.asarray(inputs["norm2_b"], np.float32)

    trivial = (
        not mlp_bias.any() and not cb.any() and not n1b.any() and not n2b.any()
        and np.all(n1w == 1.0) and np.all(n2w == 1.0)
    )
    if not trivial:
        raise NotImplementedError(
            "kernel compiled for trivial (ones/zeros) norm weights and biases")

    counts = np.stack([np.bincount(em[b], minlength=4) for b in range(B)])
    C = counts.max(0)
    off = np.concatenate([[0], np.cumsum(C)])
    T = int(off[-1])
    Tpad = ((T + 127) // 128) * 128

    nc, T_, Tpad_ = _get_program(tuple(int(c) for c in C), "v1")
    assert (T_, Tpad_) == (T, Tpad)

    # weights (shared across cores)
    WeT = np.ascontiguousarray(We.T)  # (1024, 7168)
    we_tiles = np.ascontiguousarray(
        WeT.reshape(8, 128, 56, 128).transpose(0, 2, 1, 3)).astype(BF16)
    WcT = np.ascontiguousarray(Wc.T)  # (5120, 2048)
    wc_tiles = np.ascontiguousarray(
        WcT.reshape(40, 128, 16, 128).transpose(0, 2, 1, 3)).astype(BF16)

    in_maps = []
    positions = []
    for b in range(B):
        rank = np.zeros(N, np.int64)
        cnt = np.zeros(4, np.int64)
        for i in range(N):
            e = em[b, i]
            rank[i] = cnt[e]
            cnt[e] += 1
        pos = off[em[b]] + rank  # sorted position per original token
        positions.append(pos)
        x_s = np.zeros((Tpad, DIM), np.float32)
        x_s[pos] = x[b]
        probs_s = np.zeros(Tpad, np.float32)
        probs_s[pos] = ep[b]
        in_maps.append({
            "x_s": x_s,
            "xT_s": np.ascontiguousarray(x_s.T),
            "probs_s": probs_s,
            "we_t": we_tiles,
            "wc_t": wc_tiles,
        })

    res = run_bass_kernel_spmd(nc, in_maps, list(range(B)))

    out = np.empty((B, N, DIM), np.float32)
    for b in range(B):
        out[b] = res.results[b]["outT"][:, positions[b]].T
    return out



# revision 24
# speedup vs baseline: 1.9817x; 1.9817x over previous
"""Trainium2 Bass kernel for nn_NestedParallelBlock.

Strategy: data-parallel over batch (core b <- batch b). Host sorts tokens by
expert into uniform-capacity groups (capacities = max count over the 8
batches, so the SPMD program is identical on every core); the nested feature
masks then become static K-tile / M-tile range restrictions (no mask tensors,
~47% FLOP reduction on both big matmuls). Attention is permutation-equivariant
so it runs in sorted order; dummy padding tokens are zero vectors whose k/v
rows vanish, handled exactly via a constant row-sum correction. All matmuls in
bf16 with f32 PSUM accumulation.

v2: everything stays dim-major end to end. Layernorm statistics are computed
with ones-matmuls (broadcast [128,T] mean/rstd for free) instead of
bn_stats-on-token-major, which kills all kv/xn DMA transposes; q stays in
SBUF; weights load as big per-tile slabs (1 DMA per 128-row tile); softmax
normalization is deferred out of the AV inner loop (one batched reciprocal +
per-head broadcast at the end). This keeps the PE matmul stream dense so the
HAM clock stays warm.
"""

import sys

if "/opt/trn_rl_repo" not in sys.path:
    sys.path.insert(0, "/opt/trn_rl_repo")

import numpy as np
import ml_dtypes

import concourse.bass as bass
import concourse.tile as tile
from concourse import bacc, mybir
from concourse.alu_op_type import AluOpType
from concourse.bass_utils import run_bass_kernel_spmd

BF16 = ml_dtypes.bfloat16
F32 = mybir.dt.float32
BF = mybir.dt.bfloat16
AF = mybir.ActivationFunctionType

DIM = 1024
NUM_EXPERTS = 4
NUM_HEADS = 16
DH = 64
MLP = 4096
EXPAND = 3 * DIM + MLP  # 7168
CDIM = 2 * DIM  # 2048
CKDIM = MLP + DIM  # 5120
B, N = 8, 1024
EPS = 1e-5

# expand row-tile order: kv rows, q rows, mlp rows
MORDER = list(range(8, 24)) + list(range(0, 8)) + list(range(24, 56))


def _chunks(total, step=512):
    c0 = 0
    out = []
    while c0 < total:
        out.append((c0, min(step, total - c0)))
        c0 += step
    return out


def _build(C, flags):
    """Build the SPMD Tile program. C = per-expert group capacities (len 4)."""
    off = [0]
    for c in C:
        off.append(off[-1] + c)
    T = off[-1]
    TT = (T + 127) // 128
    Tpad = TT * 128
    n_dummy = float(T - N)
    qblocks = _chunks(T)
    scale = float(DH) ** -0.5

    # contract valid-column start per cy M-tile
    # m 0..7: mlp rows (d_out >= 128(m+1)); m 8..15: attn rows (expert 3 only)
    def mstart(m):
        if m >= 8:
            return off[3]
        need = 128 * (m + 1)
        for g in range(4):
            if 256 << g >= need:
                return off[g]
        raise AssertionError

    # first group whose tokens use xn k-tile f: f < 2**g
    def g0_of(f):
        for g in range(4):
            if f < (1 << g):
                return g
        raise AssertionError

    nc = bacc.Bacc("TRN2", target_bir_lowering=False, debug=False, num_devices=8)

    xT_s = nc.dram_tensor("xT_s", [DIM, Tpad], F32, kind="ExternalInput").ap()
    probs_s = nc.dram_tensor("probs_s", [Tpad], F32, kind="ExternalInput").ap()
    we_s = nc.dram_tensor("we_s", [56, 128, 1024], BF, kind="ExternalInput").ap()
    wc_s = nc.dram_tensor("wc_s", [16, 128, 5120], BF, kind="ExternalInput").ap()
    outT = nc.dram_tensor("outT", [DIM, Tpad], F32, kind="ExternalOutput").ap()
    DEBUG = bool(flags.get("debug"))
    if DEBUG:
        kTd = nc.dram_tensor("kTd", [128, 8, T], BF, kind="ExternalOutput").ap()
        vTd = nc.dram_tensor("vTd", [128, 8, Tpad], BF, kind="ExternalOutput").ap()
        qTd = nc.dram_tensor("qTd", [128, 8, T], BF, kind="ExternalOutput").ap()
        attnTd = nc.dram_tensor("attnTd", [128, 8, T], BF, kind="ExternalOutput").ap()
        denDd = nc.dram_tensor("denDd", [128, 2, T], BF, kind="ExternalOutput").ap()
        rstd2d = nc.dram_tensor("rstd2d", [128, T], F32, kind="ExternalOutput").ap()

    xT_r = xT_s.rearrange("(f p) t -> f p t", p=128)
    outT_r = outT.rearrange("(f p) t -> f p t", p=128)

    with tile.TileContext(nc) as tc:
        # ---------------- persistent pool ----------------
        pP = tc.alloc_tile_pool(name="persist", bufs=1)
        gT = pP.tile([128, 32, T], BF)          # gelu(mlp_hidden)^T
        attnT = pP.tile([128, 8, T], BF)        # attention output^T (raw, then normalized)
        eps_t = pP.tile([128, 1], F32)
        ones1 = pP.tile([128, 128], BF)         # 1/1024 (LN1 mean matmuls)
        ones2 = pP.tile([128, 128], BF)         # 1/2048 (LN2 mean matmuls)
        nd_t = pP.tile([128, 1], F32)           # -n_dummy (denominator correction)
        ones_bc = pP.tile([128, 128], BF)       # K=1 broadcast matmul lhsT (value 1)
        nc.vector.memset(eps_t, EPS)
        nc.vector.memset(ones1, 1.0 / DIM)
        nc.vector.memset(ones2, 1.0 / CDIM)
        nc.vector.memset(nd_t, -n_dummy)
        nc.vector.memset(ones_bc, 1.0)

        # ---------------- L2: alive through attention ----------------
        pL2 = tc.alloc_tile_pool(name="l2", bufs=1)
        kT = pL2.tile([128, 8, T], BF)          # k rows, dim-major (LN2 applied in place)
        vT = pL2.tile([128, 8, Tpad], BF)       # v rows, dim-major (LN2 applied in place)
        qT = pL2.tile([128, 8, T], BF)          # q rows, dim-major
        denD = pL2.tile([128, 2, T], BF)        # dens: row 64 (h0) / 96 (h1), slot j%2

        # ---------------- LN1 ----------------
        pLN1 = tc.alloc_tile_pool(name="ln1", bufs=1)
        muB = pLN1.tile([128, Tpad], F32)
        rstdB = pLN1.tile([128, Tpad], F32)
        xnT = pLN1.tile([128, 8, T], BF)        # masked-LN(x)^T

        psStat = tc.alloc_tile_pool(name="psStat", bufs=1, space="PSUM")
        psMu = psStat.tile([128, Tpad], F32)
        psS2 = psStat.tile([128, Tpad], F32)

        pX = tc.alloc_tile_pool(name="ln1x", bufs=2)
        for f in range(8):
            xt = pX.tile([128, Tpad], F32, tag="xT")
            nc.sync.dma_start(out=xt, in_=xT_r[f])
            xc = pX.tile([128, Tpad], BF, tag="xc", bufs=1)
            nc.vector.tensor_copy(out=xc, in_=xt)
            sq = pX.tile([128, Tpad], BF, tag="sq", bufs=1)
            nc.vector.tensor_mul(out=sq, in0=xt, in1=xt)
            for (c0, cn) in _chunks(Tpad):
                nc.tensor.matmul(psMu[:, c0:c0 + cn], ones1, xc[:, c0:c0 + cn],
                                 start=(f == 0), stop=(f == 7))
                nc.tensor.matmul(psS2[:, c0:c0 + cn], ones1, sq[:, c0:c0 + cn],
                                 start=(f == 0), stop=(f == 7))
        # var = E[x^2] - mu^2 ; rstd = exp(-0.5*ln(var+eps))
        nc.vector.tensor_copy(out=muB, in_=psMu)
        nc.vector.tensor_mul(out=rstdB, in0=muB, in1=muB)
        nc.vector.tensor_sub(out=rstdB, in0=psS2, in1=rstdB)
        nc.scalar.activation(out=rstdB, in_=rstdB, func=AF.Ln, bias=eps_t)
        nc.scalar.activation(out=rstdB, in_=rstdB, func=AF.Exp, scale=-0.5)
        pX.release()
        psStat.release()

        # apply: xnT[f] = (x - mu) * rstd on the token range that uses k-tile f
        pX2 = tc.alloc_tile_pool(name="ln1apply", bufs=2)
        for f in range(8):
            t0 = off[g0_of(f)]
            xt2 = pX2.tile([128, Tpad], F32, tag="xT2")
            nc.sync.dma_start(out=xt2, in_=xT_r[f])
            nc.vector.tensor_sub(out=xnT[:, f, t0:T], in0=xt2[:, t0:T],
                                 in1=muB[:, t0:T])
            nc.vector.tensor_mul(out=xnT[:, f, t0:T], in0=xnT[:, f, t0:T],
                                 in1=rstdB[:, t0:T])
        pX2.release()

        # ---------------- expand matmul ----------------
        pW = tc.alloc_tile_pool(name="wexp", bufs=3)
        psE = tc.alloc_tile_pool(name="psE", bufs=2, space="PSUM")
        for m in MORDER:
            w = pW.tile([128, 1024], BF, tag="w")
            nc.sync.dma_start(out=w, in_=we_s[m])
            outs = {}
            for g in range(4):
                kt_g = 1 << g
                ps = psE.tile([128, C[g]], F32, tag=f"e{g}")
                for k in range(kt_g):
                    nc.tensor.matmul(
                        ps, w[:, 128 * k:128 * (k + 1)],
                        xnT[:, k, off[g]:off[g + 1]],
                        start=(k == 0), stop=(k == kt_g - 1))
                outs[g] = ps
            if 8 <= m < 24:
                dst = kT[:, m - 8, :] if m < 16 else vT[:, m - 16, :]
                for g in range(4):
                    nc.vector.tensor_copy(out=dst[:, off[g]:off[g + 1]],
                                          in_=outs[g])
            elif m < 8:
                for g in range(4):
                    nc.vector.tensor_copy(out=qT[:, m, off[g]:off[g + 1]],
                                          in_=outs[g])
            else:
                for g in range(4):
                    nc.scalar.activation(out=gT[:, m - 24, off[g]:off[g + 1]],
                                         in_=outs[g], func=AF.Gelu)
        psE.release()
        pW.release()
        pLN1.release()

        # ---------------- LN2 (stats via ones-matmuls, all dim-major) --------
        pLN2 = tc.alloc_tile_pool(name="ln2", bufs=1)
        muB2 = pLN2.tile([128, T], F32)
        rstd2B = pLN2.tile([128, T], F32)
        pSq = tc.alloc_tile_pool(name="ln2sq", bufs=2)
        psLN2 = tc.alloc_tile_pool(name="psLN2", bufs=1, space="PSUM")
        psMu2 = psLN2.tile([128, T], F32)
        psS22 = psLN2.tile([128, T], F32)
        for f in range(16):
            src_f = kT[:, f, 0:T] if f < 8 else vT[:, f - 8, 0:T]
            sq2 = pSq.tile([128, T], BF, tag="sq2")
            nc.vector.tensor_mul(out=sq2, in0=src_f, in1=src_f)
            for (c0, cn) in _chunks(T):
                nc.tensor.matmul(psMu2[:, c0:c0 + cn], ones2,
                                 src_f[:, c0:c0 + cn],
                                 start=(f == 0), stop=(f == 15))
                nc.tensor.matmul(psS22[:, c0:c0 + cn], ones2,
                                 sq2[:, c0:c0 + cn],
                                 start=(f == 0), stop=(f == 15))
        nc.vector.tensor_copy(out=muB2, in_=psMu2)
        nc.vector.tensor_mul(out=rstd2B, in0=muB2, in1=muB2)
        nc.vector.tensor_sub(out=rstd2B, in0=psS22, in1=rstd2B)
        nc.scalar.activation(out=rstd2B, in_=rstd2B, func=AF.Ln, bias=eps_t)
        nc.scalar.activation(out=rstd2B, in_=rstd2B, func=AF.Exp, scale=-0.5)
        psLN2.release()
        # apply in place (k rows f=0..7 feed S; v rows f=8..15 feed Vext)
        for f in range(16):
            dst_f = kT[:, f, 0:T] if f < 8 else vT[:, f - 8, 0:T]
            nc.vector.tensor_sub(out=dst_f, in0=dst_f, in1=muB2)
            nc.vector.tensor_mul(out=dst_f, in0=dst_f, in1=rstd2B)
        if DEBUG:
            nc.sync.dma_start(out=rstd2d, in_=rstd2B)
        pSq.release()
        pLN2.release()

        # ---------------- attention ----------------
        pAtt = tc.alloc_tile_pool(name="attn", bufs=1)
        psS = tc.alloc_tile_pool(name="psS", bufs=2, space="PSUM")
        psAV = tc.alloc_tile_pool(name="psAV", bufs=2, space="PSUM")

        for j in range(8):
            Vx = pAtt.tile([128, TT, 192], BF, tag="Vx", bufs=3)
            nc.vector.memset(Vx[:, :, 64:65], 1.0)
            nc.vector.memset(Vx[:, :, 144:176], 0.0)
            nc.vector.memset(Vx[:, :, 176:177], 1.0)
            for tt in range(TT):
                nc.sync.dma_start_transpose(
                    out=Vx[:, tt, 0:64],
                    in_=vT[0:64, j, 128 * tt:128 * (tt + 1)])
                nc.sync.dma_start_transpose(
                    out=Vx[:, tt, 80:144],
                    in_=vT[64:128, j, 128 * tt:128 * (tt + 1)])
            expS = [pAtt.tile([128, TT, T], BF, tag="expS", bufs=2, name="expS0"),
                    pAtt.tile([128, TT, T], BF, tag="expS", bufs=2, name="expS1")]
            for kt in range(TT):
                Mkt = min(128, T - 128 * kt)
                for h2 in range(2):
                    ps = psS.tile([128, T], F32, tag="S")
                    for (q0, nq) in qblocks:
                        nc.tensor.matmul(
                            ps[0:Mkt, q0:q0 + nq],
                            kT[64 * h2:64 * (h2 + 1), j, 128 * kt:128 * kt + Mkt],
                            qT[64 * h2:64 * (h2 + 1), j, q0:q0 + nq],
                            tile_position=(64 * h2, 0))
                    nc.scalar.activation(out=expS[h2][0:Mkt, kt, 0:T],
                                         in_=ps[0:Mkt, 0:T],
                                         func=AF.Exp, scale=scale)
            sl = j % 2
            for h2 in range(2):
                w0, wd = (0, 65) if h2 == 0 else (80, 97)
                dr = 64 if h2 == 0 else 96
                if h2 == 1:
                    astage = pAtt.tile([64, T], BF, tag="astage", bufs=1)
                for (q0, nq) in qblocks:
                    ps = psAV.tile([97, 512], F32, tag="AV")
                    for kt in range(TT):
                        Kkt = min(128, T - 128 * kt)
                        nc.tensor.matmul(
                            ps[0:wd, 0:nq],
                            Vx[0:Kkt, kt, w0:w0 + wd],
                            expS[h2][0:Kkt, kt, q0:q0 + nq],
                            start=(kt == 0), stop=(kt == TT - 1))
                    nc.vector.tensor_copy(
                        out=denD[dr:dr + 1, sl, q0:q0 + nq],
                        in_=ps[dr:dr + 1, 0:nq])
                    if h2 == 0:
                        nc.vector.tensor_copy(
                            out=attnT[0:64, j, q0:q0 + nq], in_=ps[0:64, 0:nq])
                    else:
                        nc.vector.tensor_copy(
                            out=astage[:, q0:q0 + nq], in_=ps[0:64, 0:nq])
                if h2 == 1:
                    nc.sync.dma_start(out=attnT[64:128, j, :], in_=astage)

            # per-pair deferred softmax normalization (off the AV critical path)
            nc.vector.tensor_scalar(
                out=denD[64:97, sl, 0:T], in0=denD[64:97, sl, 0:T],
                scalar1=nd_t[64:97, 0:1], scalar2=None, op0=AluOpType.add)
            with nc.allow_low_precision("softmax denoms are O(100), bf16 ok"):
                nc.vector.reciprocal(out=denD[64:97, sl, 0:T],
                                     in_=denD[64:97, sl, 0:T])
            for h2 in range(2):
                for (q0, nq) in qblocks:
                    rb = psAV.tile([128, 512], F32, tag="AV", name="rb")
                    nc.tensor.matmul(rb[:, 0:nq],
                                     ones_bc[64 + 32 * h2:65 + 32 * h2, :],
                                     denD[64 + 32 * h2:65 + 32 * h2, sl,
                                          q0:q0 + nq],
                                     tile_position=(64 + 32 * h2, 0))
                    nc.vector.tensor_mul(
                        out=attnT[64 * h2:64 * (h2 + 1), j, q0:q0 + nq],
                        in0=attnT[64 * h2:64 * (h2 + 1), j, q0:q0 + nq],
                        in1=rb[64 * h2:64 * (h2 + 1), 0:nq])

        if DEBUG:
            nc.sync.dma_start(out=kTd, in_=kT)
            nc.sync.dma_start(out=vTd, in_=vT)
            nc.sync.dma_start(out=qTd, in_=qT)
            nc.sync.dma_start(out=attnTd, in_=attnT)
            nc.sync.dma_start(out=denDd, in_=denD)

        psAV.release()
        psS.release()
        pAtt.release()
        pL2.release()

        # ---------------- contract + combine ----------------
        pC = tc.alloc_tile_pool(name="contr", bufs=2)
        psM = tc.alloc_tile_pool(name="psM", bufs=2, space="PSUM")
        psA2 = tc.alloc_tile_pool(name="psA2", bufs=2, space="PSUM")

        probs_b = pC.tile([128, Tpad], F32, tag="probs", bufs=1)
        nc.sync.dma_start(
            out=probs_b,
            in_=bass.AP(tensor=probs_s.tensor, offset=probs_s.offset,
                        ap=[[0, 128]] + probs_s.ap),
        )

        tA = off[3]
        La = T - tA
        for f in range(8):
            wm = pC.tile([128, 5120], BF, tag="wm")
            nc.sync.dma_start(out=wm, in_=wc_s[f])
            wa = pC.tile([128, 5120], BF, tag="wa")
            nc.sync.dma_start(out=wa, in_=wc_s[f + 8])
            tM = mstart(f)
            L = T - tM
            psm = psM.tile([128, T], F32, tag="CM")
            psa = psA2.tile([128, La], F32, tag="CA")
            for k in range(40):
                rhs = gT[:, k, :] if k < 32 else attnT[:, k - 32, :]
                lm = wm[:, 128 * k:128 * (k + 1)]
                la = wa[:, 128 * k:128 * (k + 1)]
                c0 = 0
                while c0 < L:
                    ncol = min(512, L - c0)
                    nc.tensor.matmul(psm[:, c0:c0 + ncol], lm,
                                     rhs[:, tM + c0:tM + c0 + ncol],
                                     start=(k == 0), stop=(k == 39))
                    c0 += ncol
                c0 = 0
                while c0 < La:
                    ncol = min(512, La - c0)
                    nc.tensor.matmul(psa[:, c0:c0 + ncol], la,
                                     rhs[:, tA + c0:tA + c0 + ncol],
                                     start=(k == 0), stop=(k == 39))
                    c0 += ncol
            xt = pC.tile([128, Tpad], F32, tag="xt")
            nc.sync.dma_start(out=xt, in_=xT_r[f])
            out_t = pC.tile([128, Tpad], F32, tag="out")
            nc.vector.tensor_mul(out=out_t[:, 0:T], in0=xt[:, 0:T],
                                 in1=probs_b[:, 0:T])
            nc.vector.tensor_add(out=out_t[:, 0:T], in0=out_t[:, 0:T],
                                 in1=xt[:, 0:T])
            tmp = pC.tile([128, T], F32, tag="tmp")
            nc.vector.tensor_mul(out=tmp[:, 0:L], in0=psm[:, 0:L],
                                 in1=probs_b[:, tM:T])
            nc.vector.tensor_add(out=out_t[:, tM:T], in0=out_t[:, tM:T],
                                 in1=tmp[:, 0:L])
            nc.vector.tensor_add(out=out_t[:, tA:T], in0=out_t[:, tA:T],
                                 in1=psa[:, 0:La])
            nc.sync.dma_start(out=outT_r[f][:, 0:T], in_=out_t[:, 0:T])

        psA2.release()
        psM.release()
        pC.release()
        pP.release()

    nc.compile()
    return nc, T, Tpad


_CACHE = {}


def _get_program(C, flags_key):
    key = (tuple(C), flags_key)
    if key not in _CACHE:
        _CACHE[key] = _build(list(C), {"debug": flags_key == "dbg"})
    return _CACHE[key]


def kernel(**inputs):
    x = np.ascontiguousarray(np.asarray(inputs["x"], np.float32))
    em = np.asarray(inputs["expert_mask"]).astype(np.int64)
    ep = np.ascontiguousarray(np.asarray(inputs["expert_probs"], np.float32))
    We = np.asarray(inputs["expand_weight"], np.float32)
    Wc = np.asarray(inputs["contract_weight"], np.float32)
    mlp_bias = np.asarray(inputs["mlp_bias"], np.float32)
    cb = np.asarray(inputs["contract_bias"], np.float32)
    n1w = np.asarray(inputs["norm1_w"], np.float32)
    n1b = np.asarray(inputs["norm1_b"], np.float32)
    n2w = np.asarray(inputs["norm2_w"], np.float32)
    n2b = np.asarray(inputs["norm2_b"], np.float32)

    trivial = (
        not mlp_bias.any() and not cb.any() and not n1b.any() and not n2b.any()
        and np.all(n1w == 1.0) and np.all(n2w == 1.0)
    )
    if not trivial:
        raise NotImplementedError(
            "kernel compiled for trivial (ones/zeros) norm weights and biases")

    counts = np.stack([np.bincount(em[b], minlength=4) for b in range(B)])
    C = counts.max(0)
    off = np.concatenate([[0], np.cumsum(C)])
    T = int(off[-1])
    Tpad = ((T + 127) // 128) * 128

    nc, T_, Tpad_ = _get_program(tuple(int(c) for c in C), "v2")
    assert (T_, Tpad_) == (T, Tpad)

    # weights (shared across cores): per 128-row tile, k-tiles side by side,
    # each [128 (contract dim), 128 (row dim)] pre-transposed for lhsT
    we_tiles = np.ascontiguousarray(
        We.reshape(56, 128, 8, 128).transpose(0, 3, 2, 1).reshape(
            56, 128, 1024)).astype(BF16)
    wc_tiles = np.ascontiguousarray(
        Wc.reshape(16, 128, 40, 128).transpose(0, 3, 2, 1).reshape(
            16, 128, 5120)).astype(BF16)

    in_maps = []
    positions = []
    for b in range(B):
        rank = np.zeros(N, np.int64)
        cnt = np.zeros(4, np.int64)
        for i in range(N):
            e = em[b, i]
            rank[i] = cnt[e]
            cnt[e] += 1
        pos = off[em[b]] + rank  # sorted position per original token
        positions.append(pos)
        x_s = np.zeros((Tpad, DIM), np.float32)
        x_s[pos] = x[b]
        probs_s = np.zeros(Tpad, np.float32)
        probs_s[pos] = ep[b]
        in_maps.append({
            "xT_s": np.ascontiguousarray(x_s.T),
            "probs_s": probs_s,
            "we_s": we_tiles,
            "wc_s": wc_tiles,
        })

    res = run_bass_kernel_spmd(nc, in_maps, list(range(B)))

    out = np.empty((B, N, DIM), np.float32)
    for b in range(B):
        out[b] = res.results[b]["outT"][:, positions[b]].T
    return out


# revision 27
# speedup vs baseline: 1.9956x; 1.0070x over previous
"""Trainium2 Bass kernel for nn_NestedParallelBlock.

Strategy: data-parallel over batch (core b <- batch b). Host sorts tokens by
expert into uniform-capacity groups (capacities = max count over the 8
batches, so the SPMD program is identical on every core); the nested feature
masks then become static K-tile / M-tile range restrictions (no mask tensors,
~47% FLOP reduction on both big matmuls). Attention is permutation-equivariant
so it runs in sorted order; dummy padding tokens are zero vectors whose k/v
rows vanish, handled exactly via a constant row-sum correction. All matmuls in
bf16 with f32 PSUM accumulation.

v2: everything stays dim-major end to end. Layernorm statistics are computed
with ones-matmuls (broadcast [128,T] mean/rstd for free) instead of
bn_stats-on-token-major, which kills all kv/xn DMA transposes; q stays in
SBUF; weights load as big per-tile slabs (1 DMA per 128-row tile); softmax
normalization is deferred out of the AV inner loop (one batched reciprocal +
per-head broadcast at the end). This keeps the PE matmul stream dense so the
HAM clock stays warm.
"""

import sys

if "/opt/trn_rl_repo" not in sys.path:
    sys.path.insert(0, "/opt/trn_rl_repo")

import numpy as np
import ml_dtypes

import concourse.bass as bass
import concourse.tile as tile
from concourse import bacc, mybir
from concourse.alu_op_type import AluOpType
from concourse.bass_utils import run_bass_kernel_spmd

BF16 = ml_dtypes.bfloat16
F32 = mybir.dt.float32
BF = mybir.dt.bfloat16
AF = mybir.ActivationFunctionType

DIM = 1024
NUM_EXPERTS = 4
NUM_HEADS = 16
DH = 64
MLP = 4096
EXPAND = 3 * DIM + MLP  # 7168
CDIM = 2 * DIM  # 2048
CKDIM = MLP + DIM  # 5120
B, N = 8, 1024
EPS = 1e-5

# expand row-tile order: kv rows, q rows, mlp rows
MORDER = list(range(8, 24)) + list(range(0, 8)) + list(range(24, 56))


def _chunks(total, step=512):
    c0 = 0
    out = []
    while c0 < total:
        out.append((c0, min(step, total - c0)))
        c0 += step
    return out


def _build(C, flags):
    """Build the SPMD Tile program. C = per-expert group capacities (len 4)."""
    off = [0]
    for c in C:
        off.append(off[-1] + c)
    T = off[-1]
    TT = (T + 127) // 128
    Tpad = TT * 128
    n_dummy = float(T - N)
    qblocks = _chunks(T)
    scale = float(DH) ** -0.5

    # contract valid-column start per cy M-tile
    # m 0..7: mlp rows (d_out >= 128(m+1)); m 8..15: attn rows (expert 3 only)
    def mstart(m):
        if m >= 8:
            return off[3]
        need = 128 * (m + 1)
        for g in range(4):
            if 256 << g >= need:
                return off[g]
        raise AssertionError

    # first group whose tokens use xn k-tile f: f < 2**g
    def g0_of(f):
        for g in range(4):
            if f < (1 << g):
                return g
        raise AssertionError

    nc = bacc.Bacc("TRN2", target_bir_lowering=False, debug=False, num_devices=8)

    xT_s = nc.dram_tensor("xT_s", [DIM, Tpad], F32, kind="ExternalInput").ap()
    probs_s = nc.dram_tensor("probs_s", [Tpad], F32, kind="ExternalInput").ap()
    we_s = nc.dram_tensor("we_s", [56, 128, 1024], BF, kind="ExternalInput").ap()
    wc_s = nc.dram_tensor("wc_s", [16, 128, 5120], BF, kind="ExternalInput").ap()
    outT = nc.dram_tensor("outT", [DIM, Tpad], F32, kind="ExternalOutput").ap()
    DEBUG = bool(flags.get("debug"))
    if DEBUG:
        kTd = nc.dram_tensor("kTd", [128, 8, T], BF, kind="ExternalOutput").ap()
        vTd = nc.dram_tensor("vTd", [128, 8, Tpad], BF, kind="ExternalOutput").ap()
        qTd = nc.dram_tensor("qTd", [128, 8, T], BF, kind="ExternalOutput").ap()
        attnTd = nc.dram_tensor("attnTd", [128, 8, T], BF, kind="ExternalOutput").ap()
        denDd = nc.dram_tensor("denDd", [128, T], BF, kind="ExternalOutput").ap()
        rstd2d = nc.dram_tensor("rstd2d", [128, T], F32, kind="ExternalOutput").ap()

    xT_r = xT_s.rearrange("(f p) t -> f p t", p=128)
    outT_r = outT.rearrange("(f p) t -> f p t", p=128)

    with tile.TileContext(nc) as tc:
        # ---------------- persistent pool ----------------
        pP = tc.alloc_tile_pool(name="persist", bufs=1)
        gT = pP.tile([128, 32, T], BF)          # gelu(mlp_hidden)^T
        attnT = pP.tile([128, 8, T], BF)        # attention output^T (raw, then normalized)
        eps_t = pP.tile([128, 1], F32)
        ones1 = pP.tile([128, 128], BF)         # 1/1024 (LN1 mean matmuls)
        ones2 = pP.tile([128, 128], BF)         # 1/2048 (LN2 mean matmuls)
        nd_t = pP.tile([128, 1], F32)           # -n_dummy (denominator correction)
        ones_bc = pP.tile([128, 128], BF)       # K=1 broadcast matmul lhsT (value 1)
        nc.vector.memset(eps_t, EPS)
        nc.vector.memset(ones1, 1.0 / DIM)
        nc.vector.memset(ones2, 1.0 / CDIM)
        nc.vector.memset(nd_t, -n_dummy)
        nc.vector.memset(ones_bc, 1.0)

        # ---------------- L2: alive through attention ----------------
        pL2 = tc.alloc_tile_pool(name="l2", bufs=1)
        kT = pL2.tile([128, 8, T], BF)          # k rows, dim-major (LN2 applied in place)
        vT = pL2.tile([128, 8, Tpad], BF)       # v rows, dim-major (LN2 applied in place)
        qT = pL2.tile([128, 8, T], BF)          # q rows, dim-major
        denF = pL2.tile([128, T], F32)          # raw dens: row 64 (h0) / 96 (h1)
        denD = pL2.tile([128, T], BF)           # reciprocal dens (broadcast matmul rhs)

        # ---------------- LN1 ----------------
        pLN1 = tc.alloc_tile_pool(name="ln1", bufs=1)
        muB = pLN1.tile([128, Tpad], F32)
        rstdB = pLN1.tile([128, Tpad], F32)
        xnT = pLN1.tile([128, 8, T], BF)        # masked-LN(x)^T

        psStat = tc.alloc_tile_pool(name="psStat", bufs=1, space="PSUM")
        psMu = psStat.tile([128, Tpad], F32)
        psS2 = psStat.tile([128, Tpad], F32)

        pX = tc.alloc_tile_pool(name="ln1x", bufs=2)
        for f in range(8):
            xt = pX.tile([128, Tpad], F32, tag="xT")
            nc.sync.dma_start(out=xt, in_=xT_r[f])
            xc = pX.tile([128, Tpad], BF, tag="xc", bufs=1)
            nc.vector.tensor_copy(out=xc, in_=xt)
            sq = pX.tile([128, Tpad], BF, tag="sq", bufs=1)
            nc.vector.tensor_mul(out=sq, in0=xt, in1=xt)
            for (c0, cn) in _chunks(Tpad):
                nc.tensor.matmul(psMu[:, c0:c0 + cn], ones1, xc[:, c0:c0 + cn],
                                 start=(f == 0), stop=(f == 7))
                nc.tensor.matmul(psS2[:, c0:c0 + cn], ones1, sq[:, c0:c0 + cn],
                                 start=(f == 0), stop=(f == 7))
        # var = E[x^2] - mu^2 ; rstd = exp(-0.5*ln(var+eps))
        nc.vector.tensor_copy(out=muB, in_=psMu)
        nc.vector.tensor_mul(out=rstdB, in0=muB, in1=muB)
        nc.vector.tensor_sub(out=rstdB, in0=psS2, in1=rstdB)
        nc.scalar.activation(out=rstdB, in_=rstdB, func=AF.Ln, bias=eps_t)
        nc.scalar.activation(out=rstdB, in_=rstdB, func=AF.Exp, scale=-0.5)
        pX.release()
        psStat.release()

        # apply: xnT[f] = (x - mu) * rstd on the token range that uses k-tile f
        pX2 = tc.alloc_tile_pool(name="ln1apply", bufs=2)
        for f in range(8):
            t0 = off[g0_of(f)]
            xt2 = pX2.tile([128, Tpad], F32, tag="xT2")
            nc.sync.dma_start(out=xt2, in_=xT_r[f])
            nc.vector.tensor_sub(out=xnT[:, f, t0:T], in0=xt2[:, t0:T],
                                 in1=muB[:, t0:T])
            nc.vector.tensor_mul(out=xnT[:, f, t0:T], in0=xnT[:, f, t0:T],
                                 in1=rstdB[:, t0:T])
        pX2.release()

        # ---------------- expand matmul ----------------
        pW = tc.alloc_tile_pool(name="wexp", bufs=3)
        psE = tc.alloc_tile_pool(name="psE", bufs=2, space="PSUM")
        for m in MORDER:
            w = pW.tile([128, 1024], BF, tag="w")
            nc.sync.dma_start(out=w, in_=we_s[m])
            outs = {}
            for g in range(4):
                kt_g = 1 << g
                ps = psE.tile([128, C[g]], F32, tag=f"e{g}")
                for k in range(kt_g):
                    nc.tensor.matmul(
                        ps, w[:, 128 * k:128 * (k + 1)],
                        xnT[:, k, off[g]:off[g + 1]],
                        start=(k == 0), stop=(k == kt_g - 1))
                outs[g] = ps
            if 8 <= m < 24:
                dst = kT[:, m - 8, :] if m < 16 else vT[:, m - 16, :]
                for g in range(4):
                    nc.scalar.copy(out=dst[:, off[g]:off[g + 1]], in_=outs[g])
            elif m < 8:
                for g in range(4):
                    nc.scalar.copy(out=qT[:, m, off[g]:off[g + 1]],
                                   in_=outs[g])
            else:
                for g in range(4):
                    nc.scalar.activation(out=gT[:, m - 24, off[g]:off[g + 1]],
                                         in_=outs[g], func=AF.Gelu)
        psE.release()
        pW.release()
        pLN1.release()

        # ---------------- LN2 (stats via ones-matmuls, all dim-major) --------
        pLN2 = tc.alloc_tile_pool(name="ln2", bufs=1)
        muB2 = pLN2.tile([128, T], F32)
        rstd2B = pLN2.tile([128, T], F32)
        pSq = tc.alloc_tile_pool(name="ln2sq", bufs=2)
        psLN2 = tc.alloc_tile_pool(name="psLN2", bufs=1, space="PSUM")
        psMu2 = psLN2.tile([128, T], F32)
        psS22 = psLN2.tile([128, T], F32)
        for f in range(16):
            src_f = kT[:, f, 0:T] if f < 8 else vT[:, f - 8, 0:T]
            sq2 = pSq.tile([128, T], BF, tag="sq2")
            nc.vector.tensor_mul(out=sq2, in0=src_f, in1=src_f)
            for (c0, cn) in _chunks(T):
                nc.tensor.matmul(psMu2[:, c0:c0 + cn], ones2,
                                 src_f[:, c0:c0 + cn],
                                 start=(f == 0), stop=(f == 15))
                nc.tensor.matmul(psS22[:, c0:c0 + cn], ones2,
                                 sq2[:, c0:c0 + cn],
                                 start=(f == 0), stop=(f == 15))
        nc.vector.tensor_copy(out=muB2, in_=psMu2)
        nc.vector.tensor_mul(out=rstd2B, in0=muB2, in1=muB2)
        nc.vector.tensor_sub(out=rstd2B, in0=psS22, in1=rstd2B)
        nc.scalar.activation(out=rstd2B, in_=rstd2B, func=AF.Ln, bias=eps_t)
        nc.scalar.activation(out=rstd2B, in_=rstd2B, func=AF.Exp, scale=-0.5)
        psLN2.release()
        # apply in place (k rows f=0..7 feed S; v rows f=8..15 feed Vext)
        for f in range(16):
            dst_f = kT[:, f, 0:T] if f < 8 else vT[:, f - 8, 0:T]
            nc.vector.tensor_sub(out=dst_f, in0=dst_f, in1=muB2)
            nc.vector.tensor_mul(out=dst_f, in0=dst_f, in1=rstd2B)
        if DEBUG:
            nc.sync.dma_start(out=rstd2d, in_=rstd2B)
        pSq.release()
        pLN2.release()

        # ---------------- attention ----------------
        pAtt = tc.alloc_tile_pool(name="attn", bufs=1)
        psS = tc.alloc_tile_pool(name="psS", bufs=2, space="PSUM")
        psAV = tc.alloc_tile_pool(name="psAV", bufs=2, space="PSUM")

        for j in range(8):
            Vx = pAtt.tile([128, TT, 192], BF, tag="Vx", bufs=5)
            nc.vector.memset(Vx[:, :, 64:65], 1.0)
            nc.vector.memset(Vx[:, :, 144:176], 0.0)
            nc.vector.memset(Vx[:, :, 176:177], 1.0)
            for tt in range(TT):
                nc.sync.dma_start_transpose(
                    out=Vx[:, tt, 0:64],
                    in_=vT[0:64, j, 128 * tt:128 * (tt + 1)])
                nc.sync.dma_start_transpose(
                    out=Vx[:, tt, 80:144],
                    in_=vT[64:128, j, 128 * tt:128 * (tt + 1)])
            expS = [pAtt.tile([128, TT, T], BF, tag="expS", bufs=2, name="expS0"),
                    pAtt.tile([128, TT, T], BF, tag="expS", bufs=2, name="expS1")]
            for kt in range(TT):
                Mkt = min(128, T - 128 * kt)
                for h2 in range(2):
                    ps = psS.tile([128, T], F32, tag="S")
                    for (q0, nq) in qblocks:
                        nc.tensor.matmul(
                            ps[0:Mkt, q0:q0 + nq],
                            kT[64 * h2:64 * (h2 + 1), j, 128 * kt:128 * kt + Mkt],
                            qT[64 * h2:64 * (h2 + 1), j, q0:q0 + nq],
                            tile_position=(64 * h2, 0))
                    nc.scalar.activation(out=expS[h2][0:Mkt, kt, 0:T],
                                         in_=ps[0:Mkt, 0:T],
                                         func=AF.Exp, scale=scale)
            for h2 in range(2):
                w0, wd = (0, 65) if h2 == 0 else (80, 97)
                dr = 64 if h2 == 0 else 96
                if h2 == 1:
                    astage = pAtt.tile([64, T], BF, tag="astage", bufs=1)
                for (q0, nq) in qblocks:
                    ps = psAV.tile([97, 512], F32, tag="AV")
                    for kt in range(TT):
                        Kkt = min(128, T - 128 * kt)
                        nc.tensor.matmul(
                            ps[0:wd, 0:nq],
                            Vx[0:Kkt, kt, w0:w0 + wd],
                            expS[h2][0:Kkt, kt, q0:q0 + nq],
                            start=(kt == 0), stop=(kt == TT - 1))
                    nc.vector.tensor_copy(
                        out=denF[dr - 64:dr - 63, q0:q0 + nq],
                        in_=ps[dr:dr + 1, 0:nq])
                    if h2 == 0:
                        nc.vector.tensor_copy(
                            out=attnT[0:64, j, q0:q0 + nq], in_=ps[0:64, 0:nq])
                    else:
                        nc.vector.tensor_copy(
                            out=astage[:, q0:q0 + nq], in_=ps[0:64, 0:nq])
                if h2 == 1:
                    nc.sync.dma_start(out=attnT[64:128, j, :], in_=astage)

            # per-pair deferred softmax normalization (off the AV critical path)
            nc.vector.tensor_scalar(
                out=denF[0:33, 0:T], in0=denF[0:33, 0:T],
                scalar1=nd_t[0:33, 0:1], scalar2=None, op0=AluOpType.add)
            nc.vector.reciprocal_approx_fast(out=denF[0:33, 0:T],
                                             in_=denF[0:33, 0:T])
            nc.vector.tensor_copy(out=denD[0:33, 0:T], in_=denF[0:33, 0:T])
            for h2 in range(2):
                for (q0, nq) in qblocks:
                    rb = psAV.tile([128, 512], F32, tag="AV", name="rb")
                    nc.tensor.matmul(rb[:, 0:nq],
                                     ones_bc[32 * h2:32 * h2 + 1, :],
                                     denD[32 * h2:32 * h2 + 1, q0:q0 + nq],
                                     tile_position=(32 * h2, 0))
                    nc.vector.tensor_mul(
                        out=attnT[64 * h2:64 * (h2 + 1), j, q0:q0 + nq],
                        in0=attnT[64 * h2:64 * (h2 + 1), j, q0:q0 + nq],
                        in1=rb[64 * h2:64 * (h2 + 1), 0:nq])

        if DEBUG:
            nc.sync.dma_start(out=kTd, in_=kT)
            nc.sync.dma_start(out=vTd, in_=vT)
            nc.sync.dma_start(out=qTd, in_=qT)
            nc.sync.dma_start(out=attnTd, in_=attnT)
            nc.sync.dma_start(out=denDd, in_=denD)

        psAV.release()
        psS.release()
        pAtt.release()
        pL2.release()

        # ---------------- contract + combine ----------------
        pC = tc.alloc_tile_pool(name="contr", bufs=2)
        psM = tc.alloc_tile_pool(name="psM", bufs=2, space="PSUM")
        psA2 = tc.alloc_tile_pool(name="psA2", bufs=2, space="PSUM")

        probs_b = pC.tile([128, Tpad], F32, tag="probs", bufs=1)
        nc.sync.dma_start(
            out=probs_b,
            in_=bass.AP(tensor=probs_s.tensor, offset=probs_s.offset,
                        ap=[[0, 128]] + probs_s.ap),
        )

        tA = off[3]
        La = T - tA
        for f in range(8):
            wm = pC.tile([128, 5120], BF, tag="wm")
            nc.sync.dma_start(out=wm, in_=wc_s[f])
            wa = pC.tile([128, 5120], BF, tag="wa")
            nc.sync.dma_start(out=wa, in_=wc_s[f + 8])
            tM = mstart(f)
            L = T - tM
            psm = psM.tile([128, T], F32, tag="CM")
            psa = psA2.tile([128, La], F32, tag="CA")
            for k in range(40):
                rhs = gT[:, k, :] if k < 32 else attnT[:, k - 32, :]
                lm = wm[:, 128 * k:128 * (k + 1)]
                la = wa[:, 128 * k:128 * (k + 1)]
                c0 = 0
                while c0 < L:
                    ncol = min(512, L - c0)
                    nc.tensor.matmul(psm[:, c0:c0 + ncol], lm,
                                     rhs[:, tM + c0:tM + c0 + ncol],
                                     start=(k == 0), stop=(k == 39))
                    c0 += ncol
                c0 = 0
                while c0 < La:
                    ncol = min(512, La - c0)
                    nc.tensor.matmul(psa[:, c0:c0 + ncol], la,
                                     rhs[:, tA + c0:tA + c0 + ncol],
                                     start=(k == 0), stop=(k == 39))
                    c0 += ncol
            xt = pC.tile([128, Tpad], F32, tag="xt")
            nc.sync.dma_start(out=xt, in_=xT_r[f])
            out_t = pC.tile([128, Tpad], F32, tag="out")
            nc.gpsimd.tensor_mul(out=out_t[:, 0:T], in0=xt[:, 0:T],
                                 in1=probs_b[:, 0:T])
            nc.gpsimd.tensor_add(out=out_t[:, 0:T], in0=out_t[:, 0:T],
                                 in1=xt[:, 0:T])
            tmp = pC.tile([128, T], F32, tag="tmp")
            nc.vector.tensor_mul(out=tmp[:, 0:L], in0=psm[:, 0:L],
                                 in1=probs_b[:, tM:T])
            nc.vector.tensor_add(out=out_t[:, tM:T], in0=out_t[:, tM:T],
                                 in1=tmp[:, 0:L])
            nc.vector.tensor_add(out=out_t[:, tA:T], in0=out_t[:, tA:T],
                                 in1=psa[:, 0:La])
            nc.sync.dma_start(out=outT_r[f][:, 0:T], in_=out_t[:, 0:T])

        psA2.release()
        psM.release()
        pC.release()
        pP.release()

    nc.compile()
    return nc, T, Tpad


_CACHE = {}


def _get_program(C, flags_key):
    key = (tuple(C), flags_key)
    if key not in _CACHE:
        _CACHE[key] = _build(list(C), {"debug": flags_key == "dbg"})
    return _CACHE[key]


def kernel(**inputs):
    x = np.ascontiguousarray(np.asarray(inputs["x"], np.float32))
    em = np.asarray(inputs["expert_mask"]).astype(np.int64)
    ep = np.ascontiguousarray(np.asarray(inputs["expert_probs"], np.float32))
    We = np.asarray(inputs["expand_weight"], np.float32)
    Wc = np.asarray(inputs["contract_weight"], np.float32)
    mlp_bias = np.asarray(inputs["mlp_bias"], np.float32)
    cb = np.asarray(inputs["contract_bias"], np.float32)
    n1w = np.asarray(inputs["norm1_w"], np.float32)
    n1b = np.asarray(inputs["norm1_b"], np.float32)
    n2w = np.asarray(inputs["norm2_w"], np.float32)
    n2b = np.asarray(inputs["norm2_b"], np.float32)

    trivial = (
        not mlp_bias.any() and not cb.any() and not n1b.any() and not n2b.any()
        and np.all(n1w == 1.0) and np.all(n2w == 1.0)
    )
    if not trivial:
        raise NotImplementedError(
            "kernel compiled for trivial (ones/zeros) norm weights and biases")

    counts = np.stack([np.bincount(em[b], minlength=4) for b in range(B)])
    C = counts.max(0)
    off = np.concatenate([[0], np.cumsum(C)])
    T = int(off[-1])
    Tpad = ((T + 127) // 128) * 128

    nc, T_, Tpad_ = _get_program(tuple(int(c) for c in C), "v2")
    assert (T_, Tpad_) == (T, Tpad)

    # weights (shared across cores): per 128-row tile, k-tiles side by side,
    # each [128 (contract dim), 128 (row dim)] pre-transposed for lhsT
    we_tiles = np.ascontiguousarray(
        We.reshape(56, 128, 8, 128).transpose(0, 3, 2, 1).reshape(
            56, 128, 1024)).astype(BF16)
    wc_tiles = np.ascontiguousarray(
        Wc.reshape(16, 128, 40, 128).transpose(0, 3, 2, 1).reshape(
            16, 128, 5120)).astype(BF16)

    in_maps = []
    positions = []
    for b in range(B):
        rank = np.zeros(N, np.int64)
        cnt = np.zeros(4, np.int64)
        for i in range(N):
            e = em[b, i]
            rank[i] = cnt[e]
            cnt[e] += 1
        pos = off[em[b]] + rank  # sorted position per original token
        positions.append(pos)
        x_s = np.zeros((Tpad, DIM), np.float32)
        x_s[pos] = x[b]
        probs_s = np.zeros(Tpad, np.float32)
        probs_s[pos] = ep[b]
        in_maps.append({
            "xT_s": np.ascontiguousarray(x_s.T),
            "probs_s": probs_s,
            "we_s": we_tiles,
            "wc_s": wc_tiles,
        })

    res = run_bass_kernel_spmd(nc, in_maps, list(range(B)))

    out = np.empty((B, N, DIM), np.float32)
    for b in range(B):
        out[b] = res.results[b]["outT"][:, positions[b]].T
    return out
